# revision 1
# baseline (speedup 1.0000x reference)
"""Chamfer distance kernel for Trainium2 (8 NeuronCores, SPMD).

Problem: points_src/points_trg [16, 4096, 3] f32.
  D[b,i,j] = ||x_i||^2 + ||y_j||^2 - 2 x_i.y_j
  returns (min_i D, min_j D)  — two [16, 4096] f32 arrays.

Strategy:
  - Data-parallel over batch: 2 batches per core.
  - The distance matrix tile [128 i, 512 j] is produced by ONE K=13 fp32r
    matmul: the contraction dim carries an augmented vector
      a = [x1_c, x1_c, x2_c (c=0..2), s1, s2, 1, 1]
      b = [t1_c, t2_c, t1_c (c=0..2), 1, 1, q1, q2]
    where x = x1+x2 and t = -2y = t1+t2 are 2-term splits on the fp32r
    grid (11 explicit mantissa bits, measured on HW), s = ||x||^2,
    q = ||y||^2 split likewise.  Sum_k a_k b_k = D up to ~1e-6 abs.
  - PE streams 512-wide fp32r matmuls at 1 cycle/row into PSUM (fp32).
  - ACT converts each PSUM tile to fp16 in SBUF (exact RN, verified).
  - DVE computes row-min (free-dim halving TT-min tree, finished by a
    per-batch 3D reduce) and col-min (elementwise TT-min accumulator
    over i-tiles) in fp16 2x mode.  DVE is the bottleneck engine at
    ~95% busy; cost-model wall ~326 us/core.
  - Col accumulator partition-reduce via PE transpose + DVE 3D reduce.
  - Output [128,32] blocks are DVE stream-transposed for contiguous DMA.

Numerics: outputs match the fp32 reference to ~3e-4 relative to the
output scale (dominated by the fp16 rounding of the distance values;
the matmul itself contributes ~1e-6).
"""

import sys

import numpy as np

for _p in ("/opt/trn_rl_repo",):
    if _p not in sys.path:
        sys.path.insert(0, _p)

import concourse.bass as bass
import concourse.tile as tile
from concourse import mybir
from concourse.bass_utils import run_bass_kernel_spmd

F32 = mybir.dt.float32
F32R = mybir.dt.float32r
F16 = mybir.dt.float16
MIN = mybir.AluOpType.min

B, N, C = 16, 4096, 3
NCORES = 8
BPC = B // NCORES          # batches per core
K = 13                     # augmented contraction length
NIT = N // 128             # i-tiles per batch (32)
NJC = N // 1024            # 1024-wide j-chunks per row (4)

_MAX_WAITS = 1             # this walrus build allows 1 sync wait / instruction
_DMA = "sync"            # DMA issue engine: "gpsimd" (SWDGE) or "sync" (HWDGE)
_F_WARM = True             # ACT-table warmup block
_F_CHUNK_DMA = True        # chunked input loads
_F_CHUNK_COLFOLD = True    # chunked last-i-tile colfold w/ interleaved reduce



def _split_excess_waits(nc):
    """Move excess sync waits onto same-engine NOPs placed just before."""
    for bb in nc.main_func.blocks:
        il = bb.instructions
        i = 0
        while i < len(il):
            inst = il[i]
            si = inst.sync_info
            if si is not None and si.on_wait and len(si.on_wait) > _MAX_WAITS:
                waits = list(si.on_wait)
                extra, keep = waits[:-_MAX_WAITS], waits[-_MAX_WAITS:]
                nops = []
                for k in range(0, len(extra), _MAX_WAITS):
                    chunk = extra[k:k + _MAX_WAITS]
                    nop = mybir.InstNoOp(
                        name=f"{inst.name}-wsplit{k}",
                        engine=inst.engine,
                        bass_nofuse=True,
                        sync_info=mybir.SyncInfo(on_wait=chunk, on_update=[]),
                    )
                    nc.register_instruction(nop, overwrite=True)
                    nops.append(nop)
                inst.sync_info = mybir.SyncInfo(
                    on_wait=keep, on_update=list(si.on_update))
                for j, nop in enumerate(nops):
                    il.insert(i + j, nop)
                i += len(nops)
            i += 1


def _round11(x):
    """Round to the fp32r grid: 11 explicit mantissa bits, RN."""
    x = np.asarray(x, np.float64)
    m, e = np.frexp(x)
    step = np.ldexp(1.0, e - 12)
    with np.errstate(invalid="ignore"):
        r = np.round(x / np.where(step == 0, 1.0, step)) * step
    return np.where(x == 0.0, 0.0, r)


def _build_aug(x, y):
    """Host-side augmented operands.  x,y: [B, N, 3] f32.

    Returns A, Bm: [B, K, N] f32 with all entries on the fp32r grid.
    """
    x = np.asarray(x, np.float64)
    y = np.asarray(y, np.float64)
    A = np.zeros((B, K, N), np.float64)
    Bm = np.zeros((B, K, N), np.float64)

    x1 = _round11(x)
    x2 = _round11(x - x1)
    t = -2.0 * y
    t1 = _round11(t)
    t2 = _round11(t - t1)
    for c in range(C):
        A[:, 3 * c + 0] = x1[:, :, c]
        A[:, 3 * c + 1] = x1[:, :, c]
        A[:, 3 * c + 2] = x2[:, :, c]
        Bm[:, 3 * c + 0] = t1[:, :, c]
        Bm[:, 3 * c + 1] = t2[:, :, c]
        Bm[:, 3 * c + 2] = t1[:, :, c]

    s = np.sum(x * x, axis=-1)
    s1 = _round11(s)
    s2 = _round11(s - s1)
    q = np.sum(y * y, axis=-1)
    q1 = _round11(q)
    q2 = _round11(q - q1)
    A[:, 9] = s1
    A[:, 10] = s2
    A[:, 11] = 1.0
    A[:, 12] = 1.0
    Bm[:, 9] = 1.0
    Bm[:, 10] = 1.0
    Bm[:, 11] = q1
    Bm[:, 12] = q2
    return A.astype(np.float32), Bm.astype(np.float32)


def _trace():
    """Build the SPMD per-core program.  Each core: BPC batches."""
    nc = bass.Bass()
    a_in = nc.declare_dram_parameter("a", [BPC, K, N], F32R, isOutput=False)
    b_in = nc.declare_dram_parameter("bm", [BPC, K, N], F32R, isOutput=False)
    id_in = nc.declare_dram_parameter("ident", [128, 128], F16, isOutput=False)
    omin1 = nc.declare_dram_parameter("omin1", [BPC, N], F32, isOutput=True)
    omin2 = nc.declare_dram_parameter("omin2", [BPC, N], F32, isOutput=True)

    with tile.TileContext(nc) as tc:
        with (
            tc.tile_pool(name="inp", bufs=1) as inp,
            tc.tile_pool(name="work", bufs=3) as work,
            tc.tile_pool(name="spool", bufs=4) as spool,
            tc.tile_pool(name="mm", bufs=3, space="PSUM") as mmp,
            tc.tile_pool(name="tps", bufs=2, space="PSUM") as tps,
        ):
            ident = inp.tile([128, 128], F16, tag="ident")
            NCH = 2
            CW = N // NCH
            ta, tb = [], []
            for b in range(BPC):
                t1 = inp.tile([K, N], F32R, tag=f"ta{b}")
                t2 = inp.tile([K, N], F32R, tag=f"tb{b}")
                ta.append(t1)
                tb.append(t2)
            if _F_CHUNK_DMA:
                # i-tile 0 needs only the first 128 cols of ta[0]: load them
                # first so the PE starts ~immediately, then tb[0], then ident
                # (needed late), then the rest.
                getattr(nc, _DMA).dma_start(out=ta[0][:, 0:128], in_=a_in[0][:, 0:128])
                for ch in range(NCH):
                    sl = slice(CW * ch, CW * (ch + 1))
                    getattr(nc, _DMA).dma_start(out=tb[0][:, sl], in_=b_in[0][:, sl])
                getattr(nc, _DMA).dma_start(out=ident[:], in_=id_in[:])
                if _F_WARM:
                    warm = inp.tile([128, 128], F16, tag="warm")
                    nc.scalar.copy(warm[:], ident[:])
                getattr(nc, _DMA).dma_start(out=ta[0][:, 128:N], in_=a_in[0][:, 128:N])
                for ch in range(NCH):
                    sl = slice(CW * ch, CW * (ch + 1))
                    getattr(nc, _DMA).dma_start(out=tb[1][:, sl], in_=b_in[1][:, sl])
                getattr(nc, _DMA).dma_start(out=ta[1][:], in_=a_in[1])
            else:
                getattr(nc, _DMA).dma_start(out=ident[:], in_=id_in[:])
                if _F_WARM:
                    warm = inp.tile([128, 128], F16, tag="warm")
                    nc.scalar.copy(warm[:], ident[:])
                for b in range(BPC):
                    getattr(nc, _DMA).dma_start(out=ta[b][:], in_=a_in[b])
                    getattr(nc, _DMA).dma_start(out=tb[b][:], in_=b_in[b])

            for b in range(BPC):
                G = work.tile([128, N], F16, tag="G")
                rows = work.tile([128, NIT], F32, tag="rows")
                cols = work.tile([128, NIT], F32, tag="cols")
                # per-i-tile 128-wide row-fold results, reduced in one
                # batched 3D reduce at the end of the batch
                FC = work.tile([128, NIT, 128], F16, tag="FC")
                FS = None

                prev_S = None
                for it in range(NIT):
                    lhsT = ta[b][:, 128 * it:128 * (it + 1)]
                    S = spool.tile([128, N], F16, tag="S")
                    for jc in range(NJC):
                        pm = mmp.tile([128, 1024], F32, tag="pm")
                        for h in range(2):
                            j0 = 1024 * jc + 512 * h
                            nc.tensor.matmul(
                                pm[:, 512 * h:512 * (h + 1)],
                                lhsT,
                                tb[b][:, j0:j0 + 512],
                                start=True, stop=True)
                        nc.scalar.copy(
                            S[:, 1024 * jc:1024 * (jc + 1)], pm[:])

                    # row-min: halving TT-min tree in fp16 (2x mode)
                    F2 = work.tile([128, 2048], F16, tag="F2")
                    nc.vector.tensor_tensor(F2[:], S[:, :2048], S[:, 2048:], MIN)
                    F1 = work.tile([128, 1024], F16, tag="F1")
                    nc.vector.tensor_tensor(F1[:], F2[:, :1024], F2[:, 1024:], MIN)
                    # L3 writes into an 8-i-tile staging tile; the two
                    # smallest (overhead-dominated) levels run batched 3D
                    # once per 8 i-tiles.
                    if it % 8 == 0:
                        FS = work.tile([128, 8, 512], F16, tag="FS")
                    nc.vector.tensor_tensor(
                        FS[:, it % 8, :], F1[:, :512], F1[:, 512:], MIN)
                    if it % 8 == 7:
                        FSa = work.tile([128, 8, 256], F16, tag="FSa")
                        nc.vector.tensor_tensor(
                            FSa[:], FS[:, :, 0:256], FS[:, :, 256:512], MIN)
                        nc.vector.tensor_tensor(
                            FC[:, it - 7:it + 1, :],
                            FSa[:, :, 0:128], FSa[:, :, 128:256], MIN)

                    # col-min accumulate over i-tiles.  Last i-tile goes in
                    # 512-chunks so the partition-reduce of each finished G
                    # chunk pipelines instead of waiting for the full row.
                    if it == 0:
                        prev_S = S
                    elif it == 1:
                        nc.vector.tensor_tensor(G[:], prev_S[:], S[:], MIN)
                    elif it < NIT - 1 or not _F_CHUNK_COLFOLD:
                        nc.vector.tensor_tensor(G[:], G[:], S[:], MIN)
                    else:
                        for jp in range(4):
                            sl = slice(1024 * jp, 1024 * (jp + 1))
                            nc.vector.tensor_tensor(
                                G[:, sl], G[:, sl], S[:, sl], MIN)
                            # col-min partition reduce for this chunk:
                            # 8 transposed 128x128 blocks fill one bank
                            pt = tps.tile([128, 8, 128], F16, tag="pt")
                            for k2 in range(8):
                                j0 = 1024 * jp + 128 * k2
                                nc.tensor.transpose(
                                    pt[:, k2, :], G[:, j0:j0 + 128], ident[:])
                            nc.vector.tensor_reduce(
                                cols[:, 8 * jp:8 * (jp + 1)], pt[:],
                                axis=mybir.AxisListType.X, op=MIN)
                    # first-half row-min finish off the critical tail
                    if it == NIT // 2:
                        nc.vector.tensor_reduce(
                            rows[:, :NIT // 2], FC[:, :NIT // 2, :],
                            axis=mybir.AxisListType.X, op=MIN)

                # batched row-min finish for the second half
                nc.vector.tensor_reduce(
                    rows[:, NIT // 2:], FC[:, NIT // 2:, :],
                    axis=mybir.AxisListType.X, op=MIN)

                if not _F_CHUNK_COLFOLD:
                    for jp in range(4):
                        pt = tps.tile([128, 8, 128], F16, tag="pt")
                        for k2 in range(8):
                            j0 = 1024 * jp + 128 * k2
                            nc.tensor.transpose(
                                pt[:, k2, :], G[:, j0:j0 + 128], ident[:])
                        nc.vector.tensor_reduce(
                            cols[:, 8 * jp:8 * (jp + 1)], pt[:],
                            axis=mybir.AxisListType.X, op=MIN)

                # outputs: [128, 32] where [p, q] = out[128*q + p]
                # stream-transpose 32x32 blocks then 4 contiguous DMAs
                for src, dst in ((cols, omin1), (rows, omin2)):
                    tr = work.tile([128, NIT], F32, tag="tr")
                    nc.vector.transpose(tr[:], src[:])
                    w = dst[b].rearrange("(c k) -> c k", k=128)
                    for g in range(4):
                        getattr(nc, _DMA).dma_start(
                            out=w[:, 32 * g:32 * (g + 1)],
                            in_=tr[32 * g:32 * (g + 1), :])

    _split_excess_waits(nc)
    return nc


_NC_CACHE = None


def _get_nc():
    global _NC_CACHE
    if _NC_CACHE is None:
        _NC_CACHE = _trace()
    return _NC_CACHE


def _run(points_src, points_trg, trace=False, trace_kwargs=None):
    x = np.asarray(points_src, np.float32)
    y = np.asarray(points_trg, np.float32)
    assert x.shape == (B, N, C) and y.shape == (B, N, C)
    A, Bm = _build_aug(x, y)
    ident = np.eye(128, dtype=np.float16)
    in_maps = [
        {"a": np.ascontiguousarray(A[BPC * i:BPC * (i + 1)]),
         "bm": np.ascontiguousarray(Bm[BPC * i:BPC * (i + 1)]),
         "ident": ident}
        for i in range(NCORES)
    ]
    res = run_bass_kernel_spmd(
        _get_nc(), in_maps, list(range(NCORES)), trace=trace,
        **(trace_kwargs or {}))
    min1 = np.concatenate(
        [res.results[i]["omin1"] for i in range(NCORES)], axis=0)
    min2 = np.concatenate(
        [res.results[i]["omin2"] for i in range(NCORES)], axis=0)
    return (min1, min2), res


def kernel(points_src, points_trg):
    (min1, min2), _ = _run(points_src, points_trg)
    return min1, min2



# revision 9
# speedup vs baseline: 1.1982x; 1.1982x over previous
"""Chamfer distance kernel for Trainium2 (8 NeuronCores, SPMD).

Problem: points_src/points_trg [16, 4096, 3] f32.
  D[b,i,j] = ||x_i||^2 + ||y_j||^2 - 2 x_i.y_j
  returns (min_i D, min_j D)  — two [16, 4096] f32 arrays.

Strategy (v3 — negated pipeline, 3-engine split, no transposes):
  - Data-parallel over batch: 2 batches per core.
  - The device computes NEGATED distances: the host negates the A
    operand of the K=13 augmented fp32r matmul, so PSUM holds -D and
    every min becomes a max.  Outputs are negated back on the host.
  - Per i-tile [128 i, 4096 j]: 8 fp32r matmuls into two PSUM half
    tiles [128, 2048] f32 (4 banks each, bufs=2 -> all 8 banks).
  - Readout/convert f32->f16: ACT copies both halves (1 instr each);
    every 6th i-tile DVE takes the second half instead via a fused
    tensor_scalar (PSUM f32 -> SBUF f16 copy + row-max accum in one
    1x op) to keep ACT ahead of DVE.
  - Row-max: one 4x-mode DVE tensor_scalar (bypass, accum op max)
    over the ACT-read range, accumulated into a [128,1] f32 slot; no
    reduction tree.
  - Col-max: single DVE TT-max accumulator G [128, 4096] f16 (2x).
  - Col partition-reduce: GPSIMD cross-lane tensor_reduce (axis=C,
    op=max) straight to a [1, 4096] f32 row, DMA'd to the output —
    no PE transposes, no PSUM round-trip.  Chunked after the last
    i-tile's fold so it pipelines.
"""

import sys

import numpy as np

for _p in ("/opt/trn_rl_repo",):
    if _p not in sys.path:
        sys.path.insert(0, _p)

import concourse.bass as bass
import concourse.tile as tile
from concourse import mybir
from concourse.bass_utils import run_bass_kernel_spmd

F32 = mybir.dt.float32
F32R = mybir.dt.float32r
F16 = mybir.dt.float16
MAX = mybir.AluOpType.max
BYP = mybir.AluOpType.bypass

B, N, C = 16, 4096, 3
NCORES = 8
BPC = B // NCORES          # batches per core
K = 13                     # augmented contraction length
NIT = N // 128             # i-tiles per batch (32)
HW = N // 2                # PSUM half width (2048)
QW = N // 4                # col-reduce chunk width (1024)

FX = 192                   # trailing columns DVE fuse-reads each i-tile

_MAX_WAITS = 1             # this walrus build allows 1 sync wait / instruction
_DMA = "sync"              # DMA issue engine: HWDGE via sync queue
NEG_INF = -3.0e38


def _split_excess_waits(nc):
    """Move excess sync waits onto same-engine NOPs placed just before."""
    for bb in nc.main_func.blocks:
        il = bb.instructions
        i = 0
        while i < len(il):
            inst = il[i]
            si = inst.sync_info
            if si is not None and si.on_wait and len(si.on_wait) > _MAX_WAITS:
                waits = list(si.on_wait)
                extra, keep = waits[:-_MAX_WAITS], waits[-_MAX_WAITS:]
                nops = []
                for k in range(0, len(extra), _MAX_WAITS):
                    chunk = extra[k:k + _MAX_WAITS]
                    nop = mybir.InstNoOp(
                        name=f"{inst.name}-wsplit{k}",
                        engine=inst.engine,
                        bass_nofuse=True,
                        sync_info=mybir.SyncInfo(on_wait=chunk, on_update=[]),
                    )
                    nc.register_instruction(nop, overwrite=True)
                    nops.append(nop)
                inst.sync_info = mybir.SyncInfo(
                    on_wait=keep, on_update=list(si.on_update))
                for j, nop in enumerate(nops):
                    il.insert(i + j, nop)
                i += len(nops)
            i += 1


def _round11(x):
    """Round to the fp32r grid: 11 explicit mantissa bits, RN."""
    x = np.asarray(x, np.float64)
    m, e = np.frexp(x)
    step = np.ldexp(1.0, e - 12)
    with np.errstate(invalid="ignore"):
        r = np.round(x / np.where(step == 0, 1.0, step)) * step
    return np.where(x == 0.0, 0.0, r)


def _build_aug(x, y):
    """Host-side augmented operands.  x,y: [B, N, 3] f32.

    Returns A, Bm: [B, K, N] f32 on the fp32r grid with
    sum_k A[k,i]*Bm[k,j] = -(||x_i||^2 + ||y_j||^2 - 2 x_i.y_j):
    the A side is negated so the device computes -D and reduces with
    max instead of min.
    """
    x = np.asarray(x, np.float64)
    y = np.asarray(y, np.float64)
    A = np.zeros((B, K, N), np.float64)
    Bm = np.zeros((B, K, N), np.float64)

    x1 = _round11(x)
    x2 = _round11(x - x1)
    t = -2.0 * y
    t1 = _round11(t)
    t2 = _round11(t - t1)
    for c in range(C):
        A[:, 3 * c + 0] = x1[:, :, c]
        A[:, 3 * c + 1] = x1[:, :, c]
        A[:, 3 * c + 2] = x2[:, :, c]
        Bm[:, 3 * c + 0] = t1[:, :, c]
        Bm[:, 3 * c + 1] = t2[:, :, c]
        Bm[:, 3 * c + 2] = t1[:, :, c]

    s = np.sum(x * x, axis=-1)
    s1 = _round11(s)
    s2 = _round11(s - s1)
    q = np.sum(y * y, axis=-1)
    q1 = _round11(q)
    q2 = _round11(q - q1)
    A[:, 9] = s1
    A[:, 10] = s2
    A[:, 11] = 1.0
    A[:, 12] = 1.0
    Bm[:, 9] = 1.0
    Bm[:, 10] = 1.0
    Bm[:, 11] = q1
    Bm[:, 12] = q2
    return (-A).astype(np.float32), Bm.astype(np.float32)


def _trace():
    """Build the SPMD per-core program.  Each core: BPC batches."""
    nc = bass.Bass()
    a_in = nc.declare_dram_parameter("a", [BPC, K, N], F32R, isOutput=False)
    b_in = nc.declare_dram_parameter("bm", [BPC, K, N], F32R, isOutput=False)
    omin1 = nc.declare_dram_parameter("omin1", [BPC, N], F32, isOutput=True)
    omin2 = nc.declare_dram_parameter("omin2", [BPC, N], F32, isOutput=True)

    with tile.TileContext(nc) as tc:
        with (
            tc.tile_pool(name="inp", bufs=1) as inp,
            tc.tile_pool(name="work", bufs=2) as work,
            tc.tile_pool(name="spool", bufs=3) as spool,
            tc.tile_pool(name="scr", bufs=2) as scr,
            tc.tile_pool(name="mm", bufs=2, space="PSUM") as mmp,
        ):
            NCH = 2
            CW = N // NCH
            ta, tb = [], []
            for b in range(BPC):
                t1 = inp.tile([K, N], F32R, tag=f"ta{b}")
                t2 = inp.tile([K, N], F32R, tag=f"tb{b}")
                ta.append(t1)
                tb.append(t2)
            # i-tile 0 needs only the first 128 cols of ta[0]: load them
            # first so the PE starts ~immediately, then tb[0], then the
            # rest.
            getattr(nc, _DMA).dma_start(out=ta[0][:, 0:128], in_=a_in[0][:, 0:128])
            for ch in range(NCH):
                sl = slice(CW * ch, CW * (ch + 1))
                getattr(nc, _DMA).dma_start(out=tb[0][:, sl], in_=b_in[0][:, sl])
            getattr(nc, _DMA).dma_start(out=ta[0][:, 128:N], in_=a_in[0][:, 128:N])
            for ch in range(NCH):
                sl = slice(CW * ch, CW * (ch + 1))
                getattr(nc, _DMA).dma_start(out=tb[1][:, sl], in_=b_in[1][:, sl])
            getattr(nc, _DMA).dma_start(out=ta[1][:], in_=a_in[1])

            for b in range(BPC):
                G = work.tile([128, N], F16, tag="G")
                # row-max partials: [:, it, 0] = ACT-read range,
                # [:, it, 1] = DVE-fused half (valid on fused i-tiles).
                FC = work.tile([128, NIT, 2], F32, tag="FC")
                nc.vector.memset(FC[:], NEG_INF)
                rows = work.tile([128, NIT], F32, tag="rows")
                par = work.tile([1, N], F32, tag="par")

                S0 = None
                for it in range(NIT):
                    lhsT = ta[b][:, 128 * it:128 * (it + 1)]
                    fuse = (it % FUSE_PERIOD) == (FUSE_PERIOD - 1)
                    S = spool.tile([128, N], F16, tag="S")
                    for h in range(2):
                        pm = mmp.tile([128, HW], F32, tag="pm")
                        for m in range(4):
                            j0 = HW * h + 512 * m
                            nc.tensor.matmul(
                                pm[:, 512 * m:512 * (m + 1)],
                                lhsT,
                                tb[b][:, j0:j0 + 512],
                                start=True, stop=True)
                        sl = slice(HW * h, HW * (h + 1))
                        if h == 0 or not fuse:
                            nc.scalar.copy(S[:, sl], pm[:])
                        else:
                            # fused readout: PSUM f32 -> SBUF f16 copy
                            # with row-max accumulated on the side.
                            nc.vector.tensor_scalar(
                                S[:, sl], pm[:], 0.0, None, BYP, MAX,
                                accum_out=FC[:, it, 1:2])

                    # row-max of the ACT-read range in one 4x DVE op
                    rw = HW if fuse else N
                    junk = scr.tile([128, N], F16, tag="junk")
                    nc.vector.tensor_scalar(
                        junk[:, :rw], S[:, :rw], 0.0, None, BYP, MAX,
                        accum_out=FC[:, it, 0:1])

                    # col-max accumulate on DVE
                    if it == 0:
                        S0 = S
                    elif it == 1:
                        nc.vector.tensor_tensor(G[:], S0[:], S[:], MAX)
                    elif it < NIT - 1:
                        nc.vector.tensor_tensor(G[:], G[:], S[:], MAX)
                    else:
                        # last i-tile: chunk the fold; each finished
                        # chunk goes straight into the GPSIMD cross-
                        # partition max reduce.
                        for jp in range(4):
                            sl = slice(QW * jp, QW * (jp + 1))
                            nc.vector.tensor_tensor(
                                G[:, sl], G[:, sl], S[:, sl], MAX)
                            nc.gpsimd.tensor_reduce(
                                par[:, sl], G[:, sl],
                                axis=mybir.AxisListType.C, op=MAX)
                            getattr(nc, _DMA).dma_start(
                                out=omin1[b][QW * jp:QW * (jp + 1)].rearrange(
                                    "(o k) -> o k", o=1),
                                in_=par[0:1, sl])

                # row-max partials -> negated row-min result
                nc.vector.tensor_reduce(
                    rows[:], FC[:], axis=mybir.AxisListType.X, op=MAX)
                # outputs: [128, 32] where [p, q] = out[128*q + p]
                tr = work.tile([128, NIT], F32, tag="tr")
                nc.vector.transpose(tr[:], rows[:])
                w = omin2[b].rearrange("(c k) -> c k", k=128)
                for g in range(4):
                    getattr(nc, _DMA).dma_start(
                        out=w[:, 32 * g:32 * (g + 1)],
                        in_=tr[32 * g:32 * (g + 1), :])

    _split_excess_waits(nc)
    return nc


_NC_CACHE = None


def _get_nc():
    global _NC_CACHE
    if _NC_CACHE is None:
        _NC_CACHE = _trace()
    return _NC_CACHE


def _run(points_src, points_trg, trace=False, trace_kwargs=None):
    x = np.asarray(points_src, np.float32)
    y = np.asarray(points_trg, np.float32)
    assert x.shape == (B, N, C) and y.shape == (B, N, C)
    A, Bm = _build_aug(x, y)
    in_maps = [
        {"a": np.ascontiguousarray(A[BPC * i:BPC * (i + 1)]),
         "bm": np.ascontiguousarray(Bm[BPC * i:BPC * (i + 1)])}
        for i in range(NCORES)
    ]
    res = run_bass_kernel_spmd(
        _get_nc(), in_maps, list(range(NCORES)), trace=trace,
        **(trace_kwargs or {}))
    # device computed maxes of -D: negate back to mins of D
    min1 = -np.concatenate(
        [res.results[i]["omin1"] for i in range(NCORES)], axis=0)
    min2 = -np.concatenate(
        [res.results[i]["omin2"] for i in range(NCORES)], axis=0)
    return (min1, min2), res


def kernel(points_src, points_trg):
    (min1, min2), _ = _run(points_src, points_trg)
    return min1, min2


# revision 30
# speedup vs baseline: 1.2704x; 1.0603x over previous
"""Chamfer distance kernel for Trainium2 (8 NeuronCores, SPMD).

Problem: points_src/points_trg [16, 4096, 3] f32.
  D[b,i,j] = ||x_i||^2 + ||y_j||^2 - 2 x_i.y_j
  returns (min_i D, min_j D)  — two [16, 4096] f32 arrays.

Strategy (v3 — negated pipeline, 3-engine split, no transposes):
  - Data-parallel over batch: 2 batches per core.
  - The device computes NEGATED distances: the host negates the A
    operand of the K=13 augmented fp32r matmul, so PSUM holds -D and
    every min becomes a max.  Outputs are negated back on the host.
  - Per i-tile [128 i, 4096 j]: 8 fp32r matmuls into two PSUM half
    tiles [128, 2048] f32 (4 banks each, bufs=2 -> all 8 banks).
  - Readout/convert f32->f16: ACT copies both halves (1 instr each);
    every 6th i-tile DVE takes the second half instead via a fused
    tensor_scalar (PSUM f32 -> SBUF f16 copy + row-max accum in one
    1x op) to keep ACT ahead of DVE.
  - Row-max: one 4x-mode DVE tensor_scalar (bypass, accum op max)
    over the ACT-read range, accumulated into a [128,1] f32 slot; no
    reduction tree.
  - Col-max: single DVE TT-max accumulator G [128, 4096] f16 (2x).
  - Col partition-reduce: GPSIMD cross-lane tensor_reduce (axis=C,
    op=max) straight to a [1, 4096] f32 row, DMA'd to the output —
    no PE transposes, no PSUM round-trip.  Chunked after the last
    i-tile's fold so it pipelines.
"""

import sys

import numpy as np

for _p in ("/opt/trn_rl_repo",):
    if _p not in sys.path:
        sys.path.insert(0, _p)

import concourse.bass as bass
import concourse.tile as tile
from concourse import mybir
from concourse.bass_utils import run_bass_kernel_spmd

F32 = mybir.dt.float32
F32R = mybir.dt.float32r
F16 = mybir.dt.float16
MAX = mybir.AluOpType.max
BYP = mybir.AluOpType.bypass

B, N, C = 16, 4096, 3
NCORES = 8
BPC = B // NCORES          # batches per core
K = 13                     # augmented contraction length
NIT = N // 128             # i-tiles per batch (32)
HW = N // 2                # PSUM half width (2048)
QW = N // 4                # col-reduce chunk width (1024)

FX = 128                   # trailing columns DVE fuse-reads each i-tile

_MAX_WAITS = 1             # this walrus build allows 1 sync wait / instruction
_DMA = "sync"              # DMA issue engine: HWDGE via sync queue
NEG_INF = -3.0e38


def _split_excess_waits(nc):
    """Move excess sync waits onto same-engine NOPs placed just before."""
    for bb in nc.main_func.blocks:
        il = bb.instructions
        i = 0
        while i < len(il):
            inst = il[i]
            si = inst.sync_info
            if si is not None and si.on_wait and len(si.on_wait) > _MAX_WAITS:
                waits = list(si.on_wait)
                extra, keep = waits[:-_MAX_WAITS], waits[-_MAX_WAITS:]
                nops = []
                for k in range(0, len(extra), _MAX_WAITS):
                    chunk = extra[k:k + _MAX_WAITS]
                    nop = mybir.InstNoOp(
                        name=f"{inst.name}-wsplit{k}",
                        engine=inst.engine,
                        bass_nofuse=True,
                        sync_info=mybir.SyncInfo(on_wait=chunk, on_update=[]),
                    )
                    nc.register_instruction(nop, overwrite=True)
                    nops.append(nop)
                inst.sync_info = mybir.SyncInfo(
                    on_wait=keep, on_update=list(si.on_update))
                for j, nop in enumerate(nops):
                    il.insert(i + j, nop)
                i += len(nops)
            i += 1


def _round11(x):
    """Round to the fp32r grid: 11 explicit mantissa bits, RN."""
    x = np.asarray(x, np.float64)
    m, e = np.frexp(x)
    step = np.ldexp(1.0, e - 12)
    with np.errstate(invalid="ignore"):
        r = np.round(x / np.where(step == 0, 1.0, step)) * step
    return np.where(x == 0.0, 0.0, r)


def _build_aug(x, y):
    """Host-side augmented operands.  x,y: [B, N, 3] f32.

    Returns A, Bm: [B, K, N] f32 on the fp32r grid with
    sum_k A[k,i]*Bm[k,j] = -(||x_i||^2 + ||y_j||^2 - 2 x_i.y_j):
    the A side is negated so the device computes -D and reduces with
    max instead of min.
    """
    x = np.asarray(x, np.float64)
    y = np.asarray(y, np.float64)
    A = np.zeros((B, K, N), np.float64)
    Bm = np.zeros((B, K, N), np.float64)

    x1 = _round11(x)
    x2 = _round11(x - x1)
    t = -2.0 * y
    t1 = _round11(t)
    t2 = _round11(t - t1)
    for c in range(C):
        A[:, 3 * c + 0] = x1[:, :, c]
        A[:, 3 * c + 1] = x1[:, :, c]
        A[:, 3 * c + 2] = x2[:, :, c]
        Bm[:, 3 * c + 0] = t1[:, :, c]
        Bm[:, 3 * c + 1] = t2[:, :, c]
        Bm[:, 3 * c + 2] = t1[:, :, c]

    s = np.sum(x * x, axis=-1)
    s1 = _round11(s)
    s2 = _round11(s - s1)
    q = np.sum(y * y, axis=-1)
    q1 = _round11(q)
    q2 = _round11(q - q1)
    A[:, 9] = s1
    A[:, 10] = s2
    A[:, 11] = 1.0
    A[:, 12] = 1.0
    Bm[:, 9] = 1.0
    Bm[:, 10] = 1.0
    Bm[:, 11] = q1
    Bm[:, 12] = q2
    return (-A).astype(np.float32), Bm.astype(np.float32)


def _trace():
    """Build the SPMD per-core program.  Each core: BPC batches."""
    nc = bass.Bass()
    a_in = nc.declare_dram_parameter("a", [BPC, K, N], F32R, isOutput=False)
    b_in = nc.declare_dram_parameter("bm", [BPC, K, N], F32R, isOutput=False)
    omin1 = nc.declare_dram_parameter("omin1", [BPC, N], F32, isOutput=True)
    omin2 = nc.declare_dram_parameter("omin2", [BPC, N], F32, isOutput=True)

    with tile.TileContext(nc) as tc:
        with (
            tc.tile_pool(name="inp", bufs=1) as inp,
            tc.tile_pool(name="work", bufs=2) as work,
            tc.tile_pool(name="spool", bufs=4) as spool,
            tc.tile_pool(name="scr", bufs=2) as scr,
            tc.tile_pool(name="mm", bufs=2, space="PSUM") as mmp,
        ):
            NCH = 4
            CW = N // NCH
            ta, tb = [], []
            for b in range(BPC):
                t1 = inp.tile([K, N], F32R, tag=f"ta{b}")
                t2 = inp.tile([K, N], F32R, tag=f"tb{b}")
                ta.append(t1)
                tb.append(t2)
            # Pre-ramp the PE: a few matmuls on zeroed tiles raise the
            # p-state while the input DMAs are in flight, so the first
            # real matmuls run at mid rather than low speed.
            dum = inp.tile([13, 512], F16, tag="dum")
            nc.vector.memset(dum[:], 0.0)
            rpm = mmp.tile([128, HW], F32, tag="pm")
            for r in range(4):
                nc.tensor.matmul(rpm[:, 512 * (r % 2):512 * (r % 2 + 1)],
                                 dum[:, 0:128], dum[:],
                                 start=True, stop=True)
            # i-tile 0 needs only the first 128 cols of ta[0] and the
            # first tb chunk: issue them on two different DMA queues so
            # they land in parallel and the PE starts ~immediately.
            getattr(nc, _DMA).dma_start(out=ta[0][:, 0:128], in_=a_in[0][:, 0:128])
            nc.gpsimd.dma_start(out=tb[0][:, 0:CW], in_=b_in[0][:, 0:CW])
            for ch in range(1, NCH):
                sl = slice(CW * ch, CW * (ch + 1))
                getattr(nc, _DMA).dma_start(out=tb[0][:, sl], in_=b_in[0][:, sl])
            getattr(nc, _DMA).dma_start(out=ta[0][:, 128:N], in_=a_in[0][:, 128:N])
            for ch in range(NCH):
                sl = slice(CW * ch, CW * (ch + 1))
                getattr(nc, _DMA).dma_start(out=tb[1][:, sl], in_=b_in[1][:, sl])
            getattr(nc, _DMA).dma_start(out=ta[1][:], in_=a_in[1])

            for b in range(BPC):
                G = work.tile([128, N], F16, tag="G")
                # row-max partials per i-tile: [:, it, 0] = half 0,
                # [:, it, 1] = ACT part of half 1, [:, it, 2] = the
                # DVE-fused trailing FX columns.  Slot 3 unused pad.
                FC = work.tile([128, NIT, 4], F32, tag="FC")
                nc.vector.memset(FC[:], NEG_INF)
                rows = work.tile([128, NIT], F32, tag="rows")
                par = work.tile([1, N], F32, tag="par")

                for it in range(NIT):
                    lhsT = ta[b][:, 128 * it:128 * (it + 1)]
                    last = it == NIT - 1
                    S = spool.tile([128, N], F16, tag="S")
                    junk = scr.tile([128, N], F16, tag="junk")
                    for h in range(2):
                        pm = mmp.tile([128, HW], F32, tag="pm")
                        for m in range(4):
                            j0 = HW * h + 512 * m
                            nc.tensor.matmul(
                                pm[:, 512 * m:512 * (m + 1)],
                                lhsT,
                                tb[b][:, j0:j0 + 512],
                                start=True, stop=True)
                        if h == 0:
                            if it == 0:
                                # first tile of the batch: copy in two
                                # chunks so the DVE pipeline starts a
                                # quarter earlier (row-max partial for
                                # q0 parks in the spare FC slot 3).
                                nc.scalar.copy(S[:, 0:QW], pm[:, 0:QW])
                                nc.vector.tensor_scalar(
                                    junk[:, 0:QW], S[:, 0:QW], 0.0, None,
                                    BYP, MAX, accum_out=FC[:, it, 3:4])
                                nc.vector.tensor_copy(G[:, 0:QW],
                                                      S[:, 0:QW])
                                nc.scalar.copy(S[:, QW:HW], pm[:, QW:HW])
                                nc.vector.tensor_scalar(
                                    junk[:, QW:HW], S[:, QW:HW], 0.0,
                                    None, BYP, MAX,
                                    accum_out=FC[:, it, 0:1])
                                nc.vector.tensor_copy(G[:, QW:HW],
                                                      S[:, QW:HW])
                                continue
                            nc.scalar.copy(S[:, 0:HW], pm[:])
                            if not last:
                                nc.vector.tensor_tensor(
                                    G[:, 0:HW], G[:, 0:HW], S[:, 0:HW],
                                    MAX)
                            else:
                                # final fold chunked; each chunk goes
                                # straight into the GPSIMD cross-
                                # partition max reduce.
                                for jp in range(2):
                                    sl = slice(QW * jp, QW * (jp + 1))
                                    nc.vector.tensor_tensor(
                                        G[:, sl], G[:, sl], S[:, sl], MAX)
                                    nc.gpsimd.tensor_reduce(
                                        par[:, sl], G[:, sl],
                                        axis=mybir.AxisListType.C, op=MAX)
                            # row-max of half 0 (4x) overlaps ACT's
                            # copy of half 1 (deferred on the last tile
                            # in favour of the fold chain).
                            if not last:
                                nc.vector.tensor_scalar(
                                    junk[:, 0:HW], S[:, 0:HW], 0.0, None,
                                    BYP, MAX, accum_out=FC[:, it, 0:1])
                        else:
                            nc.scalar.copy(
                                S[:, HW:N - FX], pm[:, 0:HW - FX])
                            # fused readout of the FX tail: PSUM f32 ->
                            # SBUF f16 copy + row-max accum in one op.
                            nc.vector.tensor_scalar(
                                S[:, N - FX:N], pm[:, HW - FX:HW], 0.0,
                                None, BYP, MAX, accum_out=FC[:, it, 2:3])
                            if it == 0:
                                nc.vector.tensor_copy(
                                    G[:, HW:], S[:, HW:])
                            elif not last:
                                nc.vector.tensor_tensor(
                                    G[:, HW:], G[:, HW:], S[:, HW:], MAX)
                            else:
                                # feed the cross-partition reduce ASAP;
                                # the tile's row-max runs after so the
                                # GPSIMD tail starts as early as it can.
                                for jp in range(2, 4):
                                    sl = slice(QW * jp, QW * (jp + 1))
                                    nc.vector.tensor_tensor(
                                        G[:, sl], G[:, sl], S[:, sl], MAX)
                                    nc.gpsimd.tensor_reduce(
                                        par[:, sl], G[:, sl],
                                        axis=mybir.AxisListType.C, op=MAX)
                                getattr(nc, _DMA).dma_start(
                                    out=omin1[b].rearrange(
                                        "(o k) -> o k", o=1),
                                    in_=par[0:1, :])
                            nc.vector.tensor_scalar(
                                junk[:, HW:N - FX], S[:, HW:N - FX], 0.0,
                                None, BYP, MAX, accum_out=FC[:, it, 1:2])
                    if last:
                        # row-max of half 0 was deferred on the last
                        # tile; run it now, after the fold chain.
                        nc.vector.tensor_scalar(
                            junk[:, 0:HW], S[:, 0:HW], 0.0, None,
                            BYP, MAX, accum_out=FC[:, it, 0:1])

                # row-max partials -> negated row-min result
                nc.vector.tensor_reduce(
                    rows[:], FC[:], axis=mybir.AxisListType.X, op=MAX)
                # outputs: [128, 32] where [p, q] = out[128*q + p]
                tr = work.tile([128, NIT], F32, tag="tr")
                nc.vector.transpose(tr[:], rows[:])
                w = omin2[b].rearrange("(c k) -> c k", k=128)
                for g in range(4):
                    getattr(nc, _DMA).dma_start(
                        out=w[:, 32 * g:32 * (g + 1)],
                        in_=tr[32 * g:32 * (g + 1), :])

    _split_excess_waits(nc)
    return nc


_NC_CACHE = None


def _get_nc():
    global _NC_CACHE
    if _NC_CACHE is None:
        _NC_CACHE = _trace()
    return _NC_CACHE


def _run(points_src, points_trg, trace=False, trace_kwargs=None):
    x = np.asarray(points_src, np.float32)
    y = np.asarray(points_trg, np.float32)
    assert x.shape == (B, N, C) and y.shape == (B, N, C)
    A, Bm = _build_aug(x, y)
    in_maps = [
        {"a": np.ascontiguousarray(A[BPC * i:BPC * (i + 1)]),
         "bm": np.ascontiguousarray(Bm[BPC * i:BPC * (i + 1)])}
        for i in range(NCORES)
    ]
    res = run_bass_kernel_spmd(
        _get_nc(), in_maps, list(range(NCORES)), trace=trace,
        **(trace_kwargs or {}))
    # device computed maxes of -D: negate back to mins of D
    min1 = -np.concatenate(
        [res.results[i]["omin1"] for i in range(NCORES)], axis=0)
    min2 = -np.concatenate(
        [res.results[i]["omin2"] for i in range(NCORES)], axis=0)
    return (min1, min2), res


def kernel(points_src, points_trg):
    (min1, min2), _ = _run(points_src, points_trg)
    return min1, min2


# revision 31
# speedup vs baseline: 1.2808x; 1.0081x over previous
"""Chamfer distance kernel for Trainium2 (8 NeuronCores, SPMD).

Problem: points_src/points_trg [16, 4096, 3] f32.
  D[b,i,j] = ||x_i||^2 + ||y_j||^2 - 2 x_i.y_j
  returns (min_i D, min_j D)  — two [16, 4096] f32 arrays.

Strategy (v3 — negated pipeline, 3-engine split, no transposes):
  - Data-parallel over batch: 2 batches per core.
  - The device computes NEGATED distances: the host negates the A
    operand of the K=13 augmented fp32r matmul, so PSUM holds -D and
    every min becomes a max.  Outputs are negated back on the host.
  - Per i-tile [128 i, 4096 j]: 8 fp32r matmuls into two PSUM half
    tiles [128, 2048] f32 (4 banks each, bufs=2 -> all 8 banks).
  - Readout/convert f32->f16: ACT copies both halves (1 instr each);
    every 6th i-tile DVE takes the second half instead via a fused
    tensor_scalar (PSUM f32 -> SBUF f16 copy + row-max accum in one
    1x op) to keep ACT ahead of DVE.
  - Row-max: one 4x-mode DVE tensor_scalar (bypass, accum op max)
    over the ACT-read range, accumulated into a [128,1] f32 slot; no
    reduction tree.
  - Col-max: single DVE TT-max accumulator G [128, 4096] f16 (2x).
  - Col partition-reduce: GPSIMD cross-lane tensor_reduce (axis=C,
    op=max) straight to a [1, 4096] f32 row, DMA'd to the output —
    no PE transposes, no PSUM round-trip.  Chunked after the last
    i-tile's fold so it pipelines.
"""

import sys

import numpy as np

for _p in ("/opt/trn_rl_repo",):
    if _p not in sys.path:
        sys.path.insert(0, _p)

import concourse.bass as bass
import concourse.tile as tile
from concourse import mybir
from concourse.bass_utils import run_bass_kernel_spmd

F32 = mybir.dt.float32
F32R = mybir.dt.float32r
F16 = mybir.dt.float16
MAX = mybir.AluOpType.max
BYP = mybir.AluOpType.bypass

B, N, C = 16, 4096, 3
NCORES = 8
BPC = B // NCORES          # batches per core
K = 13                     # augmented contraction length
NIT = N // 128             # i-tiles per batch (32)
HW = N // 2                # PSUM half width (2048)
QW = N // 4                # col-reduce chunk width (1024)

FX = 128                   # trailing columns DVE fuse-reads each i-tile

_MAX_WAITS = 1             # this walrus build allows 1 sync wait / instruction
_DMA = "sync"              # DMA issue engine: HWDGE via sync queue
NEG_INF = -3.0e38


def _split_excess_waits(nc):
    """Move excess sync waits onto same-engine NOPs placed just before."""
    for bb in nc.main_func.blocks:
        il = bb.instructions
        i = 0
        while i < len(il):
            inst = il[i]
            si = inst.sync_info
            if si is not None and si.on_wait and len(si.on_wait) > _MAX_WAITS:
                waits = list(si.on_wait)
                extra, keep = waits[:-_MAX_WAITS], waits[-_MAX_WAITS:]
                nops = []
                for k in range(0, len(extra), _MAX_WAITS):
                    chunk = extra[k:k + _MAX_WAITS]
                    nop = mybir.InstNoOp(
                        name=f"{inst.name}-wsplit{k}",
                        engine=inst.engine,
                        bass_nofuse=True,
                        sync_info=mybir.SyncInfo(on_wait=chunk, on_update=[]),
                    )
                    nc.register_instruction(nop, overwrite=True)
                    nops.append(nop)
                inst.sync_info = mybir.SyncInfo(
                    on_wait=keep, on_update=list(si.on_update))
                for j, nop in enumerate(nops):
                    il.insert(i + j, nop)
                i += len(nops)
            i += 1


def _round11(x):
    """Round to the fp32r grid: 11 explicit mantissa bits, RN."""
    x = np.asarray(x, np.float64)
    m, e = np.frexp(x)
    step = np.ldexp(1.0, e - 12)
    with np.errstate(invalid="ignore"):
        r = np.round(x / np.where(step == 0, 1.0, step)) * step
    return np.where(x == 0.0, 0.0, r)


def _build_aug(x, y):
    """Host-side augmented operands.  x,y: [B, N, 3] f32.

    Returns A, Bm: [B, K, N] f32 on the fp32r grid with
    sum_k A[k,i]*Bm[k,j] = -(||x_i||^2 + ||y_j||^2 - 2 x_i.y_j):
    the A side is negated so the device computes -D and reduces with
    max instead of min.
    """
    x = np.asarray(x, np.float64)
    y = np.asarray(y, np.float64)
    A = np.zeros((B, K, N), np.float64)
    Bm = np.zeros((B, K, N), np.float64)

    x1 = _round11(x)
    x2 = _round11(x - x1)
    t = -2.0 * y
    t1 = _round11(t)
    t2 = _round11(t - t1)
    for c in range(C):
        A[:, 3 * c + 0] = x1[:, :, c]
        A[:, 3 * c + 1] = x1[:, :, c]
        A[:, 3 * c + 2] = x2[:, :, c]
        Bm[:, 3 * c + 0] = t1[:, :, c]
        Bm[:, 3 * c + 1] = t2[:, :, c]
        Bm[:, 3 * c + 2] = t1[:, :, c]

    s = np.sum(x * x, axis=-1)
    s1 = _round11(s)
    s2 = _round11(s - s1)
    q = np.sum(y * y, axis=-1)
    q1 = _round11(q)
    q2 = _round11(q - q1)
    A[:, 9] = s1
    A[:, 10] = s2
    A[:, 11] = 1.0
    A[:, 12] = 1.0
    Bm[:, 9] = 1.0
    Bm[:, 10] = 1.0
    Bm[:, 11] = q1
    Bm[:, 12] = q2
    return (-A).astype(np.float32), Bm.astype(np.float32)


def _trace():
    """Build the SPMD per-core program.  Each core: BPC batches."""
    nc = bass.Bass()
    a_in = nc.declare_dram_parameter("a", [BPC, K, N], F32R, isOutput=False)
    b_in = nc.declare_dram_parameter("bm", [BPC, K, N], F32R, isOutput=False)
    omin1 = nc.declare_dram_parameter("omin1", [BPC, N], F32, isOutput=True)
    omin2 = nc.declare_dram_parameter("omin2", [BPC, N], F32, isOutput=True)

    with tile.TileContext(nc) as tc:
        with (
            tc.tile_pool(name="inp", bufs=1) as inp,
            tc.tile_pool(name="work", bufs=2) as work,
            tc.tile_pool(name="spool", bufs=4) as spool,
            tc.tile_pool(name="scr", bufs=2) as scr,
            tc.tile_pool(name="mm", bufs=2, space="PSUM") as mmp,
        ):
            NCH = 4
            CW = N // NCH
            ta, tb = [], []
            for b in range(BPC):
                t1 = inp.tile([K, N], F32R, tag=f"ta{b}")
                t2 = inp.tile([K, N], F32R, tag=f"tb{b}")
                ta.append(t1)
                tb.append(t2)
            # Pre-ramp the PE: a few matmuls on zeroed tiles raise the
            # p-state while the input DMAs are in flight, so the first
            # real matmuls run at mid rather than low speed.
            dum = inp.tile([13, 512], F16, tag="dum")
            nc.vector.memset(dum[:], 0.0)
            rpm = mmp.tile([128, HW], F32, tag="pm")
            for r in range(4):
                nc.tensor.matmul(rpm[:, 512 * (r % 2):512 * (r % 2 + 1)],
                                 dum[:, 0:128], dum[:],
                                 start=True, stop=True)
            # i-tile 0 needs only the first 128 cols of ta[0] and the
            # first tb chunk: issue them on two different DMA queues so
            # they land in parallel and the PE starts ~immediately.
            getattr(nc, _DMA).dma_start(out=ta[0][:, 0:128], in_=a_in[0][:, 0:128])
            nc.gpsimd.dma_start(out=tb[0][:, 0:CW], in_=b_in[0][:, 0:CW])
            for ch in range(1, NCH):
                sl = slice(CW * ch, CW * (ch + 1))
                getattr(nc, _DMA).dma_start(out=tb[0][:, sl], in_=b_in[0][:, sl])
            getattr(nc, _DMA).dma_start(out=ta[0][:, 128:N], in_=a_in[0][:, 128:N])
            for ch in range(NCH):
                sl = slice(CW * ch, CW * (ch + 1))
                getattr(nc, _DMA).dma_start(out=tb[1][:, sl], in_=b_in[1][:, sl])
            getattr(nc, _DMA).dma_start(out=ta[1][:], in_=a_in[1])

            for b in range(BPC):
                G = work.tile([128, N], F16, tag="G")
                # row-max partials per i-tile: [:, it, 0] = half 0,
                # [:, it, 1] = ACT part of half 1, [:, it, 2] = the
                # DVE-fused trailing FX columns.  Slot 3 unused pad.
                FC = work.tile([128, NIT, 4], F32, tag="FC")
                nc.vector.memset(FC[:], NEG_INF)
                rows = work.tile([128, NIT], F32, tag="rows")
                par = work.tile([1, N], F32, tag="par")

                for it in range(NIT):
                    lhsT = ta[b][:, 128 * it:128 * (it + 1)]
                    last = it == NIT - 1
                    S = spool.tile([128, N], F16, tag="S")
                    junk = scr.tile([128, N], F16, tag="junk")
                    for h in range(2):
                        pm = mmp.tile([128, HW], F32, tag="pm")
                        for m in range(4):
                            j0 = HW * h + 512 * m
                            nc.tensor.matmul(
                                pm[:, 512 * m:512 * (m + 1)],
                                lhsT,
                                tb[b][:, j0:j0 + 512],
                                start=True, stop=True)
                        if h == 0:
                            if it == 0:
                                # first tile of the batch: copy in two
                                # chunks so the DVE pipeline starts a
                                # quarter earlier (row-max partial for
                                # q0 parks in the spare FC slot 3).
                                nc.scalar.copy(S[:, 0:QW], pm[:, 0:QW])
                                nc.vector.tensor_scalar(
                                    junk[:, 0:QW], S[:, 0:QW], 0.0, None,
                                    BYP, MAX, accum_out=FC[:, it, 3:4])
                                nc.vector.tensor_copy(G[:, 0:QW],
                                                      S[:, 0:QW])
                                nc.scalar.copy(S[:, QW:HW], pm[:, QW:HW])
                                nc.vector.tensor_scalar(
                                    junk[:, QW:HW], S[:, QW:HW], 0.0,
                                    None, BYP, MAX,
                                    accum_out=FC[:, it, 0:1])
                                nc.vector.tensor_copy(G[:, QW:HW],
                                                      S[:, QW:HW])
                                continue
                            nc.scalar.copy(S[:, 0:HW], pm[:])
                            if not last:
                                nc.vector.tensor_tensor(
                                    G[:, 0:HW], G[:, 0:HW], S[:, 0:HW],
                                    MAX)
                            else:
                                # final fold chunked; each chunk goes
                                # straight into the GPSIMD cross-
                                # partition max reduce.
                                for jp in range(2):
                                    sl = slice(QW * jp, QW * (jp + 1))
                                    nc.vector.tensor_tensor(
                                        G[:, sl], G[:, sl], S[:, sl], MAX)
                                    nc.gpsimd.tensor_reduce(
                                        par[:, sl], G[:, sl],
                                        axis=mybir.AxisListType.C, op=MAX)
                            # row-max of half 0 (4x) overlaps ACT's
                            # copy of half 1 (deferred on the last tile
                            # in favour of the fold chain).
                            if not last:
                                nc.vector.tensor_scalar(
                                    junk[:, 0:HW], S[:, 0:HW], 0.0, None,
                                    BYP, MAX, accum_out=FC[:, it, 0:1])
                        else:
                            nc.scalar.copy(
                                S[:, HW:N - FX], pm[:, 0:HW - FX])
                            # fused readout of the FX tail: PSUM f32 ->
                            # SBUF f16 copy + row-max accum in one op.
                            nc.vector.tensor_scalar(
                                S[:, N - FX:N], pm[:, HW - FX:HW], 0.0,
                                None, BYP, MAX, accum_out=FC[:, it, 2:3])
                            if it == 0:
                                nc.vector.tensor_copy(
                                    G[:, HW:], S[:, HW:])
                            elif not last:
                                nc.vector.tensor_tensor(
                                    G[:, HW:], G[:, HW:], S[:, HW:], MAX)
                            else:
                                # feed the cross-partition reduce ASAP;
                                # the tile's row-max runs after so the
                                # GPSIMD tail starts as early as it can.
                                for jp in range(2, 4):
                                    sl = slice(QW * jp, QW * (jp + 1))
                                    nc.vector.tensor_tensor(
                                        G[:, sl], G[:, sl], S[:, sl], MAX)
                                    nc.gpsimd.tensor_reduce(
                                        par[:, sl], G[:, sl],
                                        axis=mybir.AxisListType.C, op=MAX)
                                getattr(nc, _DMA).dma_start(
                                    out=omin1[b].rearrange(
                                        "(o k) -> o k", o=1),
                                    in_=par[0:1, :])
                            nc.vector.tensor_scalar(
                                junk[:, HW:N - FX], S[:, HW:N - FX], 0.0,
                                None, BYP, MAX, accum_out=FC[:, it, 1:2])
                    if last:
                        # row-max of half 0 was deferred on the last
                        # tile; run it now, after the fold chain.
                        nc.vector.tensor_scalar(
                            junk[:, 0:HW], S[:, 0:HW], 0.0, None,
                            BYP, MAX, accum_out=FC[:, it, 0:1])

                # row-max partials -> negated row-min result
                nc.vector.tensor_reduce(
                    rows[:], FC[:], axis=mybir.AxisListType.X, op=MAX)
                # outputs: [128, 32] where [p, q] = out[128*q + p]
                # rows[p, it] -> omin2[128*it + p]: single strided DMA,
                # no transpose needed.
                getattr(nc, _DMA).dma_start(
                    out=omin2[b].rearrange("(c p) -> p c", p=128),
                    in_=rows[:])

    _split_excess_waits(nc)
    return nc


_NC_CACHE = None


def _get_nc():
    global _NC_CACHE
    if _NC_CACHE is None:
        _NC_CACHE = _trace()
    return _NC_CACHE


def _run(points_src, points_trg, trace=False, trace_kwargs=None):
    x = np.asarray(points_src, np.float32)
    y = np.asarray(points_trg, np.float32)
    assert x.shape == (B, N, C) and y.shape == (B, N, C)
    A, Bm = _build_aug(x, y)
    in_maps = [
        {"a": np.ascontiguousarray(A[BPC * i:BPC * (i + 1)]),
         "bm": np.ascontiguousarray(Bm[BPC * i:BPC * (i + 1)])}
        for i in range(NCORES)
    ]
    res = run_bass_kernel_spmd(
        _get_nc(), in_maps, list(range(NCORES)), trace=trace,
        **(trace_kwargs or {}))
    # device computed maxes of -D: negate back to mins of D
    min1 = -np.concatenate(
        [res.results[i]["omin1"] for i in range(NCORES)], axis=0)
    min2 = -np.concatenate(
        [res.results[i]["omin2"] for i in range(NCORES)], axis=0)
    return (min1, min2), res


def kernel(points_src, points_trg):
    (min1, min2), _ = _run(points_src, points_trg)
    return min1, min2
